# revision 2
# baseline (speedup 1.0000x reference)
"""Overlapping-windows unfold v6 — engine-15 rebalance, big-desc outbound,
divisor-aware inbound.

Findings stack (perfetto + DMA-only probe, this box):
  - HWDGE splits a DMA's descriptors evenly over the LARGEST DIVISOR of
    the descriptor count that is <= 16, engines 0..d-1 positionally:
    128 -> 16x8, 126 -> 14x9, 120 -> 15x8, 112 -> 16x7, 8 -> 8x1,
    127 (prime!) -> ALL on engine 0 (v4's 43us engine-0 pileup).
  - SDMA engine 15 is intrinsically ~1.19x slower on sustained outbound.
  - Small outbound descs collapse ~2x when all engines drain them
    concurrently (v3 tail); keep outbound descs >= ~40KB.
  - Per-engine drain ~27 GB/s (49.4KB desc = 1.83us).
  - SBUF APs have exactly ONE partition dim (the first).

Layout: v1's affine layout — partition p = 16b + j holds chunk j (K=125
rows) of batch b; wave src offsets affine in p since T*C = 16*K*C.

Outbound: six 128-desc passes cover rows [0,100) of every chunk; rows
[100,125) go as one 120-desc DMA (partitions 0-119 -> engines 0-14,
49.4KB descs) plus one 8-desc DMA (partitions 120-127 -> engines 0-7).
Engine row-descs: 0-7: 1025, 8-14: 1000, 15: 800 (~952 at its derate).

Inbound (Act ring): w1a cols [234,910) x 128 parts; w0 cols [0,234) x
partitions 1-126 (126 descs); w1b cols [910,1456) x 128; w2a cols
[1456,3718) x partitions 0-111 (112 descs); w2b same cols x partitions
112-126 (15 descs); p127's cols [0,234) and [1456,3484) as single-desc
edge DMAs. Zero strips issue from the sync sequencer gated on wave sems.
"""

import numpy as np

N_CTX = 9
C = 26
W = 2 * N_CTX + 1          # 19
ROWLEN = W * C             # 494
B, T = 64, 2000
N_CORES = 8
B_C = B // N_CORES
NCHUNK = 16
K = T // NCHUNK            # 125
PF = (K + 2 * N_CTX) * C   # 3718
EDGE = (K + N_CTX) * C     # 3484
HALO = N_CTX * C           # 234
W1A = 910
W1 = 1456
YFA = 25 * ROWLEN
SPANS = [(9, 12, 16), (0, 4, 9), (16, 26, 37), (37, 47, 58),
         (58, 68, 79), (79, 90, 104), (104, 113, 125)]


def _build_nc():
    import concourse.bass as bass
    import concourse.mybir as mybir

    nc = bass.Bass(target_bir_lowering=False)
    x = nc.dram_tensor("x", [B_C, T, C], mybir.dt.float32, kind="ExternalInput")
    out = nc.dram_tensor(
        "out", [B_C * T, ROWLEN], mybir.dt.float32, kind="ExternalOutput"
    )

    with (
        nc.sbuf_tensor("xs", [128, PF], mybir.dt.float32) as xs,
        nc.sbuf_tensor("ya0", [128, YFA], mybir.dt.float32) as ya0,
        nc.sbuf_tensor("ya1", [128, YFA], mybir.dt.float32) as ya1,
        nc.sbuf_tensor("ya2", [128, YFA], mybir.dt.float32) as ya2,
        nc.semaphore("w1a_sem") as w1a_sem,
        nc.semaphore("w0_sem") as w0_sem,    # w0 + et_lo: wait 32
        nc.semaphore("w1b_sem") as w1b_sem,
        nc.semaphore("w2_sem") as w2_sem,    # w2a + w2b + et_hi: wait 48
        nc.semaphore("z_sem") as z_sem,
        nc.semaphore("z2_sem") as z2_sem,
        nc.semaphore("uv_sem") as uv_sem,
        nc.semaphore("ua_sem") as ua_sem,
        nc.semaphore("oA0_sem") as oA0_sem,
        nc.semaphore("oA1_sem") as oA1_sem,
        nc.semaphore("oA2_sem") as oA2_sem,
        nc.Block() as block,
    ):
        ya = [ya0, ya1, ya2]
        oA = [oA0_sem, oA1_sem, oA2_sem]
        zeros = nc.inline_tensor(
            np.zeros(B_C * HALO, dtype=np.float32), name="zstrip"
        )

        @block.sync
        def _(sync):
            # head strips: cols [0,234) of partitions 16b, after w0+et_lo
            sync.wait_ge(w0_sem, 32)
            sync.dma_start(
                bass.AP(xs, 0, [[NCHUNK * PF, B_C], [1, HALO]]),
                bass.AP(zeros, 0, [[HALO, B_C], [1, HALO]]),
            ).then_inc(z_sem, 16)
            for i in range(3):
                r0, _, r1 = SPANS[i]
                sync.wait_ge(uv_sem, i + 1)
                sync.wait_ge(ua_sem, i + 1)
                sync.dma_start(
                    bass.AP(out, r0 * ROWLEN,
                            [[K * ROWLEN, 128], [1, (r1 - r0) * ROWLEN]]),
                    bass.AP(ya[i % 3], 0,
                            [[YFA, 128], [1, (r1 - r0) * ROWLEN]]),
                ).then_inc(oA[i % 3], 16)
            # tail strips: cols [EDGE,PF) of partitions 16b+15, after w2
            sync.wait_ge(w2_sem, 48)
            sync.dma_start(
                bass.AP(xs, 15 * PF + EDGE, [[NCHUNK * PF, B_C], [1, HALO]]),
                bass.AP(zeros, 0, [[HALO, B_C], [1, HALO]]),
            ).then_inc(z2_sem, 16)
            for i in range(3, 7):
                r0, _, r1 = SPANS[i]
                sync.wait_ge(uv_sem, i + 1)
                sync.wait_ge(ua_sem, i + 1)
                sync.dma_start(
                    bass.AP(out, r0 * ROWLEN,
                            [[K * ROWLEN, 128], [1, (r1 - r0) * ROWLEN]]),
                    bass.AP(ya[i % 3], 0,
                            [[YFA, 128], [1, (r1 - r0) * ROWLEN]]),
                ).then_inc(oA[i % 3], 16)
            sync.wait_ge(oA0_sem, 48)
            sync.wait_ge(oA1_sem, 32)
            sync.wait_ge(oA2_sem, 32)

        @block.scalar
        def _(scalar):
            # dummy 1-elem copy: ACT identity table preloads while waves
            # dispatch/drain (yd[0,0] is rewritten by cpB before use)
            scalar.copy(
                bass.AP(ya2, 0, [[YFA, 1], [1, 1]]),
                bass.AP(xs, 0, [[PF, 1], [1, 1]]),
            )
            # w1a: cols [234,910) x all 128 partitions
            scalar.dma_start(
                bass.AP(xs, HALO, [[PF, 128], [1, W1A - HALO]]),
                bass.AP(x, 0, [[K * C, 128], [1, W1A - HALO]]),
            ).then_inc(w1a_sem, 16)
            # w0: cols [0,234) x partitions 1-126 (126 descs -> 14 engines)
            scalar.dma_start(
                bass.AP(xs, PF, [[PF, 126], [1, HALO]]),
                bass.AP(x, K * C - HALO, [[K * C, 126], [1, HALO]]),
            ).then_inc(w0_sem, 16)
            # et_lo: p127 cols [0,234)
            scalar.dma_start(
                bass.AP(xs, 127 * PF, [[PF, 1], [1, HALO]]),
                bass.AP(x, 127 * K * C - HALO, [[HALO, 1], [1, HALO]]),
            ).then_inc(w0_sem, 16)
            # w1b: cols [910,1456) x all 128
            scalar.dma_start(
                bass.AP(xs, W1A, [[PF, 128], [1, W1 - W1A]]),
                bass.AP(x, W1A - HALO, [[K * C, 128], [1, W1 - W1A]]),
            ).then_inc(w1b_sem, 16)
            # ACT half of pass-0 copy, before the remaining dispatches
            scalar.wait_ge(w1a_sem, 16)
            scalar.copy(
                bass.AP(ya0, 3 * ROWLEN, [[YFA, 128], [ROWLEN, 4], [1, ROWLEN]]),
                bass.AP(xs, 12 * C, [[PF, 128], [C, 4], [1, ROWLEN]]),
            ).then_inc(ua_sem, 1)
            # et_hi: p127 cols [1456,3484)
            scalar.dma_start(
                bass.AP(xs, 127 * PF + W1, [[PF, 1], [1, EDGE - W1]]),
                bass.AP(x, 127 * K * C - HALO + W1,
                        [[EDGE - W1, 1], [1, EDGE - W1]]),
            ).then_inc(w2_sem, 16)
            # w2a: cols [1456,3718) x partitions 0-111 (112 -> 16 engines)
            scalar.dma_start(
                bass.AP(xs, W1, [[PF, 112], [1, PF - W1]]),
                bass.AP(x, W1 - HALO, [[K * C, 112], [1, PF - W1]]),
            ).then_inc(w2_sem, 16)
            # w2b: cols [1456,3718) x partitions 112-126 (15 -> 15 engines)
            scalar.dma_start(
                bass.AP(xs, 112 * PF + W1, [[PF, 15], [1, PF - W1]]),
                bass.AP(x, 112 * K * C - HALO + W1, [[K * C, 15], [1, PF - W1]]),
            ).then_inc(w2_sem, 16)
            # remaining ACT copy halves
            for i, (r0, rv, r1) in list(enumerate(SPANS))[1:]:
                if i == 1:
                    scalar.wait_ge(z_sem, 16)
                if i == 2:
                    scalar.wait_ge(w1b_sem, 16)
                if i == 3:
                    scalar.wait_ge(w2_sem, 48)
                if i == 6:
                    scalar.wait_ge(z2_sem, 16)
                if i >= 3:
                    scalar.wait_ge(oA[i % 3], 16 if i < 6 else 32)
                scalar.copy(
                    bass.AP(ya[i % 3], (rv - r0) * ROWLEN,
                            [[YFA, 128], [ROWLEN, r1 - rv], [1, ROWLEN]]),
                    bass.AP(xs, rv * C,
                            [[PF, 128], [C, r1 - rv], [1, ROWLEN]]),
                ).then_inc(ua_sem, 1)


        @block.vector
        def _(vector):
            for i, (r0, rv, r1) in enumerate(SPANS):
                if i == 0:
                    vector.wait_ge(w1a_sem, 16)
                if i == 1:
                    vector.wait_ge(z_sem, 16)
                if i == 2:
                    vector.wait_ge(w1b_sem, 16)
                if i == 3:
                    vector.wait_ge(w2_sem, 48)
                if i == 6:
                    vector.wait_ge(z2_sem, 16)
                if i >= 3:
                    vector.wait_ge(oA[i % 3], 16 if i < 6 else 32)
                vector.tensor_copy(
                    bass.AP(ya[i % 3], 0,
                            [[YFA, 128], [ROWLEN, rv - r0], [1, ROWLEN]]),
                    bass.AP(xs, r0 * C,
                            [[PF, 128], [C, rv - r0], [1, ROWLEN]]),
                ).then_inc(uv_sem, 1)


    return nc


def kernel(x: np.ndarray) -> np.ndarray:
    from concourse.bass_utils import run_bass_kernel_spmd

    x = np.ascontiguousarray(np.asarray(x), dtype=np.float32)
    assert x.shape == (B, T, C), x.shape

    nc = _build_nc()
    in_maps = [{"x": x[i * B_C : (i + 1) * B_C]} for i in range(N_CORES)]
    res = run_bass_kernel_spmd(nc, in_maps, core_ids=list(range(N_CORES)))
    return np.concatenate([r["out"] for r in res.results], axis=0)
